# revision 17
# baseline (speedup 1.0000x reference)
# kernel.py — Trainium2 Bass kernel for nn_DSQGAttentionTCR
# Gated causal attention, T5 relative-position banded bias, KV injection,
# per-head gain, qkv/gate/out projections.
#
# Sharding: 8 cores = (batch b in 0..3) x (head-group hg in 0..1, 4 heads each).
# Each core computes its 768 qkv projection columns, causal attention for its
# 4 heads, the gate slice, and a partial out-projection; host sums the two
# partials per batch and adds out_b.
#
# Self-contained: shapes/sharding hardcoded; no sibling imports.

import numpy as np
import ml_dtypes

import concourse.bass as bass
import concourse.mybir as mybir
import concourse.tile as tile
from concourse import bacc
from concourse.bass_utils import run_bass_kernel_spmd
import bass_rust

F32 = mybir.dt.float32
F32R = mybir.dt.float32r
BF16 = mybir.dt.bfloat16
AF = mybir.ActivationFunctionType
ALU = mybir.AluOpType

B, N, D, H = 4, 2048, 512, 8
HD = 64
HG = 4              # heads per core
NCORES = 8
QBW = 256           # q-block width in attention
NQB = N // QBW      # 8 q-blocks
SLOTS = 4           # j-tiles per score-psum group
ETAB_U = 2176       # E-table u extent (offset 128)
ETAB_OFF = 128

# T5 bucket starts for buckets 28..43 (derived from the jnp reference formula,
# N=2048, MAX_EXACT=28, NUM_BUCKETS=44)
_BSTARTS = np.array([28, 33, 42, 55, 72, 94, 123, 161, 210, 274,
                     359, 469, 613, 801, 1048, 1370], dtype=np.int64)
_BAND_RANGES = [range(38, 44), range(34, 40), range(28, 35), range(20, 28),
                range(13, 21), range(7, 14), range(3, 9), range(0, 6)]

TRACE = False
LAST_RESULT = None
_CACHED_NC = None
PHASES = 3   # debug: 1 = projections only, 2 = +attention, 3 = full


def _bucket_of(d):
    d = np.asarray(d)
    return np.where(d < 28, d, 28 + np.searchsorted(_BSTARTS, d, side='right') - 1)


def _band_mask():
    m = np.zeros((44, H), dtype=np.float32)
    for h, band in enumerate(_BAND_RANGES):
        for j in band:
            m[j, h] = 1.0
    return m


def _prep_cores(inputs):
    """Host-side sharding/layout prep. Returns list of 8 in_maps."""
    x = np.ascontiguousarray(np.asarray(inputs["x"], dtype=np.float32))
    k_delta = np.asarray(inputs["k_delta"], dtype=np.float32)
    v_delta = np.asarray(inputs["v_delta"], dtype=np.float32)
    qkv_w = np.asarray(inputs["qkv_w"], dtype=np.float32)
    qkv_b = np.asarray(inputs["qkv_b"], dtype=np.float32)
    gate_w = np.asarray(inputs["gate_w"], dtype=np.float32)
    gate_b = np.asarray(inputs["gate_b"], dtype=np.float32)
    out_w = np.asarray(inputs["out_w"], dtype=np.float32)
    pos_bias = np.asarray(inputs["pos_bias"], dtype=np.float32)
    if_gain = np.asarray(inputs["if_gain"], dtype=np.float32)

    band = _band_mask()
    # Out-of-band bias is -100 in the reference, which softmax renormalizes
    # away for fully-out-of-band rows. exp(-100) underflows bf16, so use -30:
    # softmax-invariant for fully-out-of-band rows, and the relative weight of
    # out-of-band terms in mixed rows stays ~1e-13 (vs 1e-43), both below
    # tolerance. exp(-30)=9.4e-14 is representable in bf16.
    eff_bias = pos_bias * band + (1.0 - band) * (-30.0)    # [44, H]

    # E tables per head: E[h][p, u] = exp(bias(d)) with d = u - ETAB_OFF - p,
    # 0 for d < 0 (causal).
    p = np.arange(128)[:, None]
    u = np.arange(ETAB_U)[None, :]
    dmat = u - ETAB_OFF - p                              # [-255, 2047]
    dcl = np.clip(dmat, 0, N - 1)
    buck = _bucket_of(dcl)                               # [128, ETAB_U]
    etab_all = []
    for h in range(H):
        E = np.exp(eff_bias[buck, h].astype(np.float32))
        E = np.where(dmat >= 0, E, 0.0).astype(np.float32)
        etab_all.append(E.astype(ml_dtypes.bfloat16))
    etab_all = np.stack(etab_all)                        # [H, 128, ETAB_U] bf16

    scale = np.float32(1.0 / np.sqrt(HD))

    in_maps = []
    for c in range(NCORES):
        b, hg = c // 2, c % 2
        hs = slice(4 * hg, 4 * hg + 4)          # global heads of this core
        cs = slice(256 * hg, 256 * hg + 256)    # their 256 feature columns

        gain_cols = np.repeat(if_gain[hs], HD)  # [256]

        wq = np.ascontiguousarray(qkv_w[:, cs] * scale)
        wk = np.ascontiguousarray(qkv_w[:, 512 + 256 * hg: 512 + 256 * hg + 256])
        wv = np.ascontiguousarray(
            qkv_w[:, 1024 + 256 * hg: 1024 + 256 * hg + 256] * gain_cols[None, :])
        wg = np.ascontiguousarray(gate_w[:, cs])
        wo = np.ascontiguousarray(out_w[256 * hg: 256 * hg + 256, :].astype(ml_dtypes.bfloat16))

        bq = np.ascontiguousarray(qkv_b[cs] * scale)
        bk = qkv_b[512 + 256 * hg: 512 + 256 * hg + 256]
        bv = (qkv_b[1024 + 256 * hg: 1024 + 256 * hg + 256]) * gain_cols
        bg = np.ascontiguousarray(gate_b[cs])

        # kdT [128, 2, 2048]: row r of pair pr -> head hl=2pr+(r>=64), hd=r%64
        kdT = np.empty((128, 2, N), dtype=np.float32)
        for pr in range(2):
            for pe in range(2):
                hl = 2 * pr + pe
                h = 4 * hg + hl
                # [N, HD] -> [HD, N]
                kdT[64 * pe:64 * pe + 64, pr, :] = (
                    k_delta[b, h].T + bk[64 * hl:64 * hl + 64][:, None])

        # vd [2048, 256] q-major: col 64*hl+hd
        vdl = v_delta[b, hs]                    # [4, N, HD]
        vd = (np.transpose(vdl, (1, 0, 2)).reshape(N, 256) * gain_cols[None, :]
              + bv[None, :]).astype(np.float32)
        vd = np.ascontiguousarray(vd)

        etab = np.ascontiguousarray(etab_all[hs])   # [4, 128, ETAB_U] bf16

        in_maps.append({
            "xT": np.ascontiguousarray(x[b].T),
            "wq": wq, "wk": wk, "wv": wv, "wg": wg, "wo": wo,
            "bq": bq, "bg": bg,
            "kdT": np.ascontiguousarray(kdT),
            "vd": vd,
            "etab": etab,
            "ident": np.eye(64, dtype=np.float32),
        })
    return in_maps


def _eap(t_ap, offset, ap_pairs):
    """Construct a raw (possibly overlapping) AP on tensor of t_ap."""
    return bass_rust.AP(tensor=t_ap.tensor, offset=offset, ap=ap_pairs)


def _build_nc():
    nc = bacc.Bacc("TRN2", target_bir_lowering=False, debug=False)

    xT_d = nc.dram_tensor("xT", [D, N], F32R, kind="ExternalInput").ap()
    wq_d = nc.dram_tensor("wq", [D, 256], F32R, kind="ExternalInput").ap()
    wk_d = nc.dram_tensor("wk", [D, 256], F32R, kind="ExternalInput").ap()
    wv_d = nc.dram_tensor("wv", [D, 256], F32R, kind="ExternalInput").ap()
    wg_d = nc.dram_tensor("wg", [D, 256], F32R, kind="ExternalInput").ap()
    wo_d = nc.dram_tensor("wo", [256, D], BF16, kind="ExternalInput").ap()
    bq_d = nc.dram_tensor("bq", [256], F32, kind="ExternalInput").ap()
    bg_d = nc.dram_tensor("bg", [256], F32, kind="ExternalInput").ap()
    kdT_d = nc.dram_tensor("kdT", [128, 2, N], F32, kind="ExternalInput").ap()
    vd_d = nc.dram_tensor("vd", [N, 256], F32, kind="ExternalInput").ap()
    etab_d = nc.dram_tensor("etab", [HG, 128, ETAB_U], BF16,
                            kind="ExternalInput").ap()
    ident_d = nc.dram_tensor("ident", [64, 64], F32, kind="ExternalInput").ap()
    y_d = nc.dram_tensor("y", [N, D], F32, kind="ExternalOutput").ap()

    with tile.TileContext(nc) as tc:
        _kernel_body(nc, tc, xT_d, wq_d, wk_d, wv_d, wg_d, wo_d, bq_d, bg_d,
                     kdT_d, vd_d, etab_d, ident_d, y_d)
    nc.compile()
    return nc


def _kernel_body(nc, tc, xT_d, wq_d, wk_d, wv_d, wg_d, wo_d, bq_d, bg_d,
                 kdT_d, vd_d, etab_d, ident_d, y_d):
    from contextlib import ExitStack

    from concourse import library_config

    with ExitStack() as stk:
        per = stk.enter_context(tc.tile_pool(name="per", bufs=1))

        # ---- persistent tiles ----
        qT = per.tile([128, 2, N], BF16, tag="qT")
        kT = per.tile([128, 2, N], BF16, tag="kT")
        vv = per.tile([128, HG, 16, 68], BF16, tag="vv")
        gsb = per.tile([64, HG, N], BF16, tag="gsb")
        zt = per.tile([64, HG, N], BF16, tag="zt")
        wo_sb = per.tile([64, HG, D], BF16, tag="wo_sb")
        etab_sb = per.tile([128, HG, ETAB_U], BF16, tag="etab_sb")
        lpack = per.tile([128, N], F32, tag="lpack")
        rcol_sb = per.tile([128, 16, HG], F32, tag="rcol_sb")
        id_sb = per.tile([128, 64], F32, tag="id_sb")

        nc.sync.dma_start(id_sb[64:128, :], ident_d)
        # rows 64..67 of lpack accumulate per-head softmax denominators
        # (each od contributes its own head's row; the others are zero);
        # rows 68..127 are only read by the transpose, keep them finite
        nc.vector.memset(lpack[64:128, :], 1.0)
        nc.vector.memset(lpack[64:68, :], 0.0)

        nc.sync.dma_start(wo_sb[:, :, :], wo_d.rearrange("(hc p) f -> p hc f", p=64))
        nc.sync.dma_start(etab_sb[:, :, :], etab_d.rearrange("h p u -> p h u"))

        # ================= phase 1: projections =================
        with tc.tile_pool(name="p1", bufs=1) as p1, \
             tc.tile_pool(name="p1ps", bufs=4, space="PSUM") as pp1:
            xT = p1.tile([128, 4, N], F32R, tag="xT")
            wq_sb = p1.tile([128, 4, 256], F32R, tag="wq_sb")
            wk_sb = p1.tile([128, 4, 256], F32R, tag="wk_sb")
            wv_sb = p1.tile([128, 4, 256], F32R, tag="wv_sb")
            wg_sb = p1.tile([128, 4, 256], F32R, tag="wg_sb")
            kdT_sb = p1.tile([128, 2, N], F32, tag="kdT_sb")
            vd_sb = p1.tile([128, 16, 256], F32, tag="vd_sb")
            gtmp = p1.tile([128, 2, N], BF16, tag="gtmp")
            bq_sb = p1.tile([128, 2], F32, tag="bq_sb")
            bg_sb = p1.tile([128, 2], F32, tag="bg_sb")

            nc.sync.dma_start(xT[:, :, :], xT_d.rearrange("(kc p) n -> p kc n", p=128))
            nc.sync.dma_start(wq_sb[:, :, :], wq_d.rearrange("(kc p) m -> p kc m", p=128))
            nc.sync.dma_start(wk_sb[:, :, :], wk_d.rearrange("(kc p) m -> p kc m", p=128))
            nc.sync.dma_start(wv_sb[:, :, :], wv_d.rearrange("(kc p) m -> p kc m", p=128))
            nc.sync.dma_start(wg_sb[:, :, :], wg_d.rearrange("(kc p) m -> p kc m", p=128))
            nc.sync.dma_start(kdT_sb[:, :, :], kdT_d)
            nc.sync.dma_start(vd_sb[:, :, :], vd_d.rearrange("(po p) c -> p po c", p=128))
            nc.sync.dma_start(bq_sb[:, :], bq_d.rearrange("(c p) -> p c", p=128))
            nc.sync.dma_start(bg_sb[:, :], bg_d.rearrange("(c p) -> p c", p=128))

            # staggered ones columns: head hl's denominator row lands on
            # output partition 64+hl of the AV matmul
            nc.vector.memset(vv[:, :, :, 64:68], 0.0)
            for hl in range(HG):
                nc.vector.memset(vv[:, hl, :, 64 + hl:65 + hl], 1.0)

            # Q, K, G projections: feature-major outputs [128, 2, N]
            for pr in range(2):
                for nb in range(4):
                    nbs = slice(512 * nb, 512 * nb + 512)
                    psq = pp1.tile([128, 512], F32, tag="ps_proj")
                    psk = pp1.tile([128, 512], F32, tag="ps_proj")
                    psg = pp1.tile([128, 512], F32, tag="ps_proj")
                    for kc in range(4):
                        nc.tensor.matmul(
                            psq[:, :], wq_sb[:, kc, 128 * pr:128 * pr + 128],
                            xT[:, kc, nbs],
                            start=(kc == 0), stop=(kc == 3))
                    for kc in range(4):
                        nc.tensor.matmul(
                            psk[:, :], wk_sb[:, kc, 128 * pr:128 * pr + 128],
                            xT[:, kc, nbs],
                            start=(kc == 0), stop=(kc == 3))
                    for kc in range(4):
                        nc.tensor.matmul(
                            psg[:, :], wg_sb[:, kc, 128 * pr:128 * pr + 128],
                            xT[:, kc, nbs],
                            start=(kc == 0), stop=(kc == 3))
                    nc.vector.tensor_scalar_add(qT[:, pr, nbs], psq[:, :],
                                                bq_sb[:, pr:pr + 1])
                    nc.vector.tensor_add(kT[:, pr, nbs], psk[:, :], kdT_sb[:, pr, nbs])
                    nc.scalar.activation(gtmp[:, pr, nbs], psg[:, :], AF.Sigmoid,
                                         bias=bg_sb[:, pr:pr + 1])

            # V projection: q-major [n, 256] tiles, evict into vv (+ delta)
            for nt in range(16):
                nts = slice(128 * nt, 128 * nt + 128)
                psv = pp1.tile([128, 256], F32, tag="ps_v")
                for kc in range(4):
                    nc.tensor.matmul(
                        psv[:, :], xT[:, kc, nts],
                        wv_sb[:, kc, :],
                        start=(kc == 0), stop=(kc == 3))
                nc.vector.tensor_add(
                    vv[:, :, nt, 0:64],
                    psv.rearrange("p (h c) -> p h c", c=64),
                    vd_sb[:, nt, :].rearrange("p (h c) -> p h c", c=64))

            # gate partition shuffle: [128, 2, N] -> [64, 4, N]
            for hl in range(4):
                nc.sync.dma_start(gsb[0:64, hl, :],
                                  gtmp[64 * (hl % 2):64 * (hl % 2) + 64, hl // 2, :])

        # ================= phase 2: attention =================
        if PHASES < 2:
            return
        with tc.tile_pool(name="sa", bufs=1, space="PSUM") as sap, \
             tc.tile_pool(name="sb", bufs=1, space="PSUM") as sbp, \
             tc.tile_pool(name="ov", bufs=4, space="PSUM") as ovp, \
             tc.tile_pool(name="ev", bufs=2) as evp, \
             tc.tile_pool(name="pt", bufs=3) as ptp:

            for pr in range(2):
                for qb in range(NQB):
                    J = 2 * qb + 2
                    qs = slice(QBW * qb, QBW * qb + QBW)
                    # separate accumulation tiles per (head parity, j-half):
                    # concurrent PE array tiles must write different PSUM banks
                    od = {}
                    av_n = {}
                    for pe in range(2):
                        for hf in range(2):
                            od[pe, hf] = ovp.tile([128, QBW], F32, tag="ov",
                                                  name=f"ov{pe}{hf}")
                            av_n[pe, hf] = 0

                    ngroups = (J + SLOTS - 1) // SLOTS
                    for g in range(ngroups):
                        j0 = g * SLOTS
                        L = min(SLOTS, J - j0)
                        jd_hi = j0 + L - 1
                        sps = {0: sap.tile([128, SLOTS, QBW], F32, tag="sa", name="sA"),
                               1: sbp.tile([128, SLOTS, QBW], F32, tag="sb", name="sB")}
                        # scores: slot s holds jd = jd_hi - s (so the E-table
                        # group view has a positive slot stride)
                        for s in range(L):
                            jd = jd_hi - s
                            js = slice(128 * jd, 128 * jd + 128)
                            for pe in range(2):
                                hf = pe
                                nc.tensor.matmul(
                                    sps[pe][:, s, :],
                                    kT[64 * hf:64 * hf + 64, pr, js],
                                    qT[64 * hf:64 * hf + 64, pr, qs],
                                    start=True, stop=True)
                        # exp + E multiply (grouped)
                        pts = {}
                        for pe in range(2):
                            hl = 2 * pr + pe
                            pt = ptp.tile([128, SLOTS, QBW], BF16, tag="pt", name=f"pt{pe}")
                            pts[pe] = pt
                            nc.scalar.activation(pt[:, :L, :], sps[pe][:, :L, :],
                                                 AF.Exp)
                            u0 = ETAB_OFF + QBW * qb - 128 * jd_hi
                            eview = _eap(etab_sb, hl * ETAB_U + u0,
                                         [[HG * ETAB_U, 128], [128, L], [1, QBW]])
                            nc.vector.tensor_tensor(pt[:, :L, :], pt[:, :L, :],
                                                    eview, ALU.mult)
                        # AV: each (head, j-half) accumulates into its own
                        # bank; the two halves run on different PE array tiles
                        for s in range(L):
                            jd = jd_hi - s
                            for pe in range(2):
                                hl = 2 * pr + pe
                                for hf in range(2):
                                    i = av_n[pe, hf]
                                    av_n[pe, hf] += 1
                                    nc.tensor.matmul(
                                        od[pe, hf][0:68, :],
                                        vv[64 * hf:64 * hf + 64, hl, jd, 0:68],
                                        pts[pe][64 * hf:64 * hf + 64, s, :],
                                        start=(i == 0),
                                        stop=(i == J - 1))

                    # evict: combine halves, stash the denominator rows,
                    # gate-multiply the (unnormalized) o into the z tile;
                    # division by l happens at the out-projection eviction
                    for pe in range(2):
                        hl = 2 * pr + pe
                        t = evp.tile([64, QBW], F32, tag="ev")
                        nc.vector.tensor_copy(t[0:64, :], od[pe, 0][0:64, :])
                        nc.vector.tensor_add(lpack[64:68, qs],
                                             lpack[64:68, qs],
                                             od[pe, 0][64:68, :])
                        nc.vector.tensor_add(lpack[64:68, qs],
                                             lpack[64:68, qs],
                                             od[pe, 1][64:68, :])
                        nc.vector.scalar_tensor_tensor(
                            t[0:64, :], od[pe, 1][0:64, :], 1.0, t[0:64, :],
                            op0=ALU.mult, op1=ALU.add)
                        nc.vector.tensor_mul(zt[0:64, hl, qs], t[0:64, :],
                                             gsb[0:64, hl, qs])

        # ================= phase 3: out projection =================
        if PHASES < 3:
            return
        # y[n] = sum_h (z~_h @ wo_h) / l_h[n]; the division applies at psum
        # eviction with per-partition reciprocal columns obtained by PE-
        # transposing the packed l rows.
        with tc.tile_pool(name="yps", bufs=4, space="PSUM") as yp, \
             tc.tile_pool(name="lps", bufs=2, space="PSUM") as lpp, \
             tc.tile_pool(name="ysb", bufs=4) as ysp:
            for nt in range(16):
                nts = slice(128 * nt, 128 * nt + 128)
                lps = lpp.tile([128, 64], F32, tag="lps")
                nc.tensor.transpose(lps[:, :], lpack[64:128, nts],
                                    id_sb[64:128, :])
                nc.vector.reciprocal(rcol_sb[:, nt, :], lps[:, 0:HG])
                ya = ysp.tile([128, D], F32, tag="ya")
                yb = ysp.tile([128, D], F32, tag="yb")
                for hl in range(HG):
                    psy = yp.tile([128, D], F32, tag="psy", name=f"psy{hl}")
                    nc.tensor.matmul(psy[:, :], zt[0:64, hl, nts],
                                     wo_sb[0:64, hl, :],
                                     start=True, stop=True)
                    r = rcol_sb[:, nt, hl:hl + 1]
                    if hl == 0:
                        nc.scalar.activation(ya[:, :], psy[:, :], AF.Copy,
                                             scale=r)
                    elif hl == 1:
                        nc.vector.scalar_tensor_tensor(
                            ya[:, :], psy[:, :], r, ya[:, :],
                            op0=ALU.mult, op1=ALU.add)
                    elif hl == 2:
                        nc.scalar.activation(yb[:, :], psy[:, :], AF.Copy,
                                             scale=r)
                    else:
                        nc.vector.scalar_tensor_tensor(
                            yb[:, :], psy[:, :], r, yb[:, :],
                            op0=ALU.mult, op1=ALU.add)
                nc.sync.dma_start(y_d[nts, :], ya[:, :])
                nc.gpsimd.dma_start(y_d[nts, :], yb[:, :], accum_op=ALU.add)


def kernel(**inputs):
    global LAST_RESULT, _CACHED_NC
    in_maps = _prep_cores(inputs)
    if _CACHED_NC is None:
        _CACHED_NC = _build_nc()
    nc = _CACHED_NC
    res = run_bass_kernel_spmd(nc, in_maps, core_ids=list(range(NCORES)),
                               trace=TRACE)
    LAST_RESULT = res
    out_b = np.asarray(inputs["out_b"], dtype=np.float32)
    full = np.empty((B, N, D), dtype=np.float32)
    for b in range(B):
        full[b] = res.results[2 * b]["y"] + res.results[2 * b + 1]["y"] + out_b
    return full


# revision 18
# speedup vs baseline: 1.2339x; 1.2339x over previous
# kernel.py — Trainium2 Bass kernel for nn_DSQGAttentionTCR
# Gated causal attention, T5 relative-position banded bias, KV injection,
# per-head gain, qkv/gate/out projections.
#
# Sharding: 8 cores = (batch b in 0..3) x (head-group hg in 0..1, 4 heads each).
# Each core computes its 768 qkv projection columns, causal attention for its
# 4 heads, the gate slice, and a partial out-projection; host sums the two
# partials per batch and adds out_b.
#
# Self-contained: shapes/sharding hardcoded; no sibling imports.

import numpy as np
import ml_dtypes

import concourse.bass as bass
import concourse.mybir as mybir
import concourse.tile as tile
from concourse import bacc
from concourse.bass_utils import run_bass_kernel_spmd
import bass_rust

F32 = mybir.dt.float32
F32R = mybir.dt.float32r
BF16 = mybir.dt.bfloat16
AF = mybir.ActivationFunctionType
ALU = mybir.AluOpType

B, N, D, H = 4, 2048, 512, 8
HD = 64
HG = 4              # heads per core
NCORES = 8
QBW = 256           # q-block width in attention
NQB = N // QBW      # 8 q-blocks
SLOTS = 6           # j-tiles per score-psum group
ETAB_U = 2176       # E-table u extent (offset 128)
ETAB_OFF = 128

# T5 bucket starts for buckets 28..43 (derived from the jnp reference formula,
# N=2048, MAX_EXACT=28, NUM_BUCKETS=44)
_BSTARTS = np.array([28, 33, 42, 55, 72, 94, 123, 161, 210, 274,
                     359, 469, 613, 801, 1048, 1370], dtype=np.int64)
_BAND_RANGES = [range(38, 44), range(34, 40), range(28, 35), range(20, 28),
                range(13, 21), range(7, 14), range(3, 9), range(0, 6)]

TRACE = False
LAST_RESULT = None
_CACHED_NC = None
PHASES = 3   # debug: 1 = projections only, 2 = +attention, 3 = full


def _bucket_of(d):
    d = np.asarray(d)
    return np.where(d < 28, d, 28 + np.searchsorted(_BSTARTS, d, side='right') - 1)


def _band_mask():
    m = np.zeros((44, H), dtype=np.float32)
    for h, band in enumerate(_BAND_RANGES):
        for j in band:
            m[j, h] = 1.0
    return m


def _prep_cores(inputs):
    """Host-side sharding/layout prep. Returns list of 8 in_maps."""
    x = np.ascontiguousarray(np.asarray(inputs["x"], dtype=np.float32))
    k_delta = np.asarray(inputs["k_delta"], dtype=np.float32)
    v_delta = np.asarray(inputs["v_delta"], dtype=np.float32)
    qkv_w = np.asarray(inputs["qkv_w"], dtype=np.float32)
    qkv_b = np.asarray(inputs["qkv_b"], dtype=np.float32)
    gate_w = np.asarray(inputs["gate_w"], dtype=np.float32)
    gate_b = np.asarray(inputs["gate_b"], dtype=np.float32)
    out_w = np.asarray(inputs["out_w"], dtype=np.float32)
    pos_bias = np.asarray(inputs["pos_bias"], dtype=np.float32)
    if_gain = np.asarray(inputs["if_gain"], dtype=np.float32)

    band = _band_mask()
    # Out-of-band bias is -100 in the reference, which softmax renormalizes
    # away for fully-out-of-band rows. exp(-100) underflows bf16, so use -30:
    # softmax-invariant for fully-out-of-band rows, and the relative weight of
    # out-of-band terms in mixed rows stays ~1e-13 (vs 1e-43), both below
    # tolerance. exp(-30)=9.4e-14 is representable in bf16.
    eff_bias = pos_bias * band + (1.0 - band) * (-30.0)    # [44, H]

    # E tables per head: E[h][p, u] = exp(bias(d)) with d = u - ETAB_OFF - p,
    # 0 for d < 0 (causal).
    p = np.arange(128)[:, None]
    u = np.arange(ETAB_U)[None, :]
    dmat = u - ETAB_OFF - p                              # [-255, 2047]
    dcl = np.clip(dmat, 0, N - 1)
    buck = _bucket_of(dcl)                               # [128, ETAB_U]
    etab_all = []
    for h in range(H):
        E = np.exp(eff_bias[buck, h].astype(np.float32))
        E = np.where(dmat >= 0, E, 0.0).astype(np.float32)
        etab_all.append(E.astype(ml_dtypes.bfloat16))
    etab_all = np.stack(etab_all)                        # [H, 128, ETAB_U] bf16

    scale = np.float32(1.0 / np.sqrt(HD))

    in_maps = []
    for c in range(NCORES):
        b, hg = c // 2, c % 2
        hs = slice(4 * hg, 4 * hg + 4)          # global heads of this core
        cs = slice(256 * hg, 256 * hg + 256)    # their 256 feature columns

        gain_cols = np.repeat(if_gain[hs], HD)  # [256]

        wq = np.ascontiguousarray(qkv_w[:, cs] * scale)
        wk = np.ascontiguousarray(qkv_w[:, 512 + 256 * hg: 512 + 256 * hg + 256])
        wv = np.ascontiguousarray(
            qkv_w[:, 1024 + 256 * hg: 1024 + 256 * hg + 256] * gain_cols[None, :])
        wg = np.ascontiguousarray(gate_w[:, cs])
        wo = np.ascontiguousarray(out_w[256 * hg: 256 * hg + 256, :].astype(ml_dtypes.bfloat16))

        bq = np.ascontiguousarray(qkv_b[cs] * scale)
        bk = qkv_b[512 + 256 * hg: 512 + 256 * hg + 256]
        bv = (qkv_b[1024 + 256 * hg: 1024 + 256 * hg + 256]) * gain_cols
        bg = np.ascontiguousarray(gate_b[cs])

        # kdT [128, 2, 2048]: row r of pair pr -> head hl=2pr+(r>=64), hd=r%64
        kdT = np.empty((128, 2, N), dtype=np.float32)
        for pr in range(2):
            for pe in range(2):
                hl = 2 * pr + pe
                h = 4 * hg + hl
                # [N, HD] -> [HD, N]
                kdT[64 * pe:64 * pe + 64, pr, :] = (
                    k_delta[b, h].T + bk[64 * hl:64 * hl + 64][:, None])

        # vd [2048, 256] q-major: col 64*hl+hd
        vdl = v_delta[b, hs]                    # [4, N, HD]
        vd = (np.transpose(vdl, (1, 0, 2)).reshape(N, 256) * gain_cols[None, :]
              + bv[None, :]).astype(np.float32)
        vd = np.ascontiguousarray(vd)

        etab = np.ascontiguousarray(etab_all[hs])   # [4, 128, ETAB_U] bf16

        in_maps.append({
            "xT": np.ascontiguousarray(x[b].T),
            "wq": wq, "wk": wk, "wv": wv, "wg": wg, "wo": wo,
            "bq": bq, "bg": bg,
            "kdT": np.ascontiguousarray(kdT),
            "vd": vd,
            "etab": etab,
            "ident": np.eye(64, dtype=np.float32),
        })
    return in_maps


def _eap(t_ap, offset, ap_pairs):
    """Construct a raw (possibly overlapping) AP on tensor of t_ap."""
    return bass_rust.AP(tensor=t_ap.tensor, offset=offset, ap=ap_pairs)


def _build_nc():
    nc = bacc.Bacc("TRN2", target_bir_lowering=False, debug=False)

    xT_d = nc.dram_tensor("xT", [D, N], F32R, kind="ExternalInput").ap()
    wq_d = nc.dram_tensor("wq", [D, 256], F32R, kind="ExternalInput").ap()
    wk_d = nc.dram_tensor("wk", [D, 256], F32R, kind="ExternalInput").ap()
    wv_d = nc.dram_tensor("wv", [D, 256], F32R, kind="ExternalInput").ap()
    wg_d = nc.dram_tensor("wg", [D, 256], F32R, kind="ExternalInput").ap()
    wo_d = nc.dram_tensor("wo", [256, D], BF16, kind="ExternalInput").ap()
    bq_d = nc.dram_tensor("bq", [256], F32, kind="ExternalInput").ap()
    bg_d = nc.dram_tensor("bg", [256], F32, kind="ExternalInput").ap()
    kdT_d = nc.dram_tensor("kdT", [128, 2, N], F32, kind="ExternalInput").ap()
    vd_d = nc.dram_tensor("vd", [N, 256], F32, kind="ExternalInput").ap()
    etab_d = nc.dram_tensor("etab", [HG, 128, ETAB_U], BF16,
                            kind="ExternalInput").ap()
    ident_d = nc.dram_tensor("ident", [64, 64], F32, kind="ExternalInput").ap()
    y_d = nc.dram_tensor("y", [N, D], F32, kind="ExternalOutput").ap()

    with tile.TileContext(nc) as tc:
        _kernel_body(nc, tc, xT_d, wq_d, wk_d, wv_d, wg_d, wo_d, bq_d, bg_d,
                     kdT_d, vd_d, etab_d, ident_d, y_d)
    nc.compile()
    return nc


def _kernel_body(nc, tc, xT_d, wq_d, wk_d, wv_d, wg_d, wo_d, bq_d, bg_d,
                 kdT_d, vd_d, etab_d, ident_d, y_d):
    from contextlib import ExitStack

    from concourse import library_config

    with ExitStack() as stk:
        per = stk.enter_context(tc.tile_pool(name="per", bufs=1))

        # ---- persistent tiles ----
        # per-head K-padded layouts: rows 0..63 hold the head's 64 features,
        # rows 64..127 are zero so score matmuls contract over K=128 without
        # mixing heads (keeps one PE tiling mode for the whole kernel)
        qT = per.tile([128, HG, N], BF16, tag="qT")
        kT = per.tile([128, HG, N], BF16, tag="kT")
        vv = per.tile([128, HG, 16, 68], BF16, tag="vv")
        gsb = per.tile([64, HG, N], BF16, tag="gsb")
        zt = per.tile([64, HG, N], BF16, tag="zt")
        wo_sb = per.tile([64, HG, D], BF16, tag="wo_sb")
        etab_sb = per.tile([128, HG, ETAB_U], BF16, tag="etab_sb")
        lpack = per.tile([128, N], F32, tag="lpack")
        rcol_sb = per.tile([128, 16, HG], F32, tag="rcol_sb")
        id_sb = per.tile([128, 64], F32, tag="id_sb")

        nc.sync.dma_start(id_sb[64:128, :], ident_d)
        # rows 64..67 of lpack accumulate per-head softmax denominators
        # (each od contributes its own head's row; the others are zero);
        # rows 68..127 are only read by the transpose, keep them finite
        nc.vector.memset(lpack[64:128, :], 1.0)
        nc.vector.memset(lpack[64:68, :], 0.0)

        nc.sync.dma_start(wo_sb[:, :, :], wo_d.rearrange("(hc p) f -> p hc f", p=64))
        nc.sync.dma_start(etab_sb[:, :, :], etab_d.rearrange("h p u -> p h u"))

        # ================= phase 1: projections =================
        with tc.tile_pool(name="p1", bufs=1) as p1, \
             tc.tile_pool(name="p1ps", bufs=4, space="PSUM") as pp1:
            xT = p1.tile([128, 4, N], F32R, tag="xT")
            wq_sb = p1.tile([128, 4, 256], F32R, tag="wq_sb")
            wk_sb = p1.tile([128, 4, 256], F32R, tag="wk_sb")
            wv_sb = p1.tile([128, 4, 256], F32R, tag="wv_sb")
            wg_sb = p1.tile([128, 4, 256], F32R, tag="wg_sb")
            qTp = p1.tile([128, 2, N], BF16, tag="qTp")
            kTp = p1.tile([128, 2, N], BF16, tag="kTp")
            kdT_sb = p1.tile([128, 2, N], F32, tag="kdT_sb")
            vd_sb = p1.tile([128, 16, 256], F32, tag="vd_sb")
            gtmp = p1.tile([128, 2, N], BF16, tag="gtmp")
            bq_sb = p1.tile([128, 2], F32, tag="bq_sb")
            bg_sb = p1.tile([128, 2], F32, tag="bg_sb")

            nc.sync.dma_start(xT[:, :, :], xT_d.rearrange("(kc p) n -> p kc n", p=128))
            nc.sync.dma_start(wq_sb[:, :, :], wq_d.rearrange("(kc p) m -> p kc m", p=128))
            nc.sync.dma_start(wk_sb[:, :, :], wk_d.rearrange("(kc p) m -> p kc m", p=128))
            nc.sync.dma_start(wv_sb[:, :, :], wv_d.rearrange("(kc p) m -> p kc m", p=128))
            nc.sync.dma_start(wg_sb[:, :, :], wg_d.rearrange("(kc p) m -> p kc m", p=128))
            nc.sync.dma_start(kdT_sb[:, :, :], kdT_d)
            nc.sync.dma_start(vd_sb[:, :, :], vd_d.rearrange("(po p) c -> p po c", p=128))
            nc.sync.dma_start(bq_sb[:, :], bq_d.rearrange("(c p) -> p c", p=128))
            nc.sync.dma_start(bg_sb[:, :], bg_d.rearrange("(c p) -> p c", p=128))

            nc.vector.memset(qT[64:128, :, :], 0.0)
            nc.vector.memset(kT[64:128, :, :], 0.0)
            # staggered ones columns: head hl's denominator row lands on
            # output partition 64+hl of the AV matmul
            nc.vector.memset(vv[:, :, :, 64:68], 0.0)
            for hl in range(HG):
                nc.vector.memset(vv[:, hl, :, 64 + hl:65 + hl], 1.0)

            # Q, K, G projections: feature-major outputs [128, 2, N]
            for pr in range(2):
                for nb in range(4):
                    nbs = slice(512 * nb, 512 * nb + 512)
                    psq = pp1.tile([128, 512], F32, tag="ps_proj")
                    psk = pp1.tile([128, 512], F32, tag="ps_proj")
                    psg = pp1.tile([128, 512], F32, tag="ps_proj")
                    for kc in range(4):
                        nc.tensor.matmul(
                            psq[:, :], wq_sb[:, kc, 128 * pr:128 * pr + 128],
                            xT[:, kc, nbs],
                            start=(kc == 0), stop=(kc == 3))
                    for kc in range(4):
                        nc.tensor.matmul(
                            psk[:, :], wk_sb[:, kc, 128 * pr:128 * pr + 128],
                            xT[:, kc, nbs],
                            start=(kc == 0), stop=(kc == 3))
                    for kc in range(4):
                        nc.tensor.matmul(
                            psg[:, :], wg_sb[:, kc, 128 * pr:128 * pr + 128],
                            xT[:, kc, nbs],
                            start=(kc == 0), stop=(kc == 3))
                    nc.vector.tensor_scalar_add(qTp[:, pr, nbs], psq[:, :],
                                                bq_sb[:, pr:pr + 1])
                    nc.vector.tensor_add(kTp[:, pr, nbs], psk[:, :], kdT_sb[:, pr, nbs])
                    nc.scalar.activation(gtmp[:, pr, nbs], psg[:, :], AF.Sigmoid,
                                         bias=bg_sb[:, pr:pr + 1])

            # V projection: q-major [n, 256] tiles, evict into vv (+ delta)
            for nt in range(16):
                nts = slice(128 * nt, 128 * nt + 128)
                psv = pp1.tile([128, 256], F32, tag="ps_v")
                for kc in range(4):
                    nc.tensor.matmul(
                        psv[:, :], xT[:, kc, nts],
                        wv_sb[:, kc, :],
                        start=(kc == 0), stop=(kc == 3))
                nc.vector.tensor_add(
                    vv[:, :, nt, 0:64],
                    psv.rearrange("p (h c) -> p h c", c=64),
                    vd_sb[:, nt, :].rearrange("p (h c) -> p h c", c=64))

            # partition shuffles: pair layout [128, 2, N] -> per-head [64|128, 4, N]
            for hl in range(4):
                sl = slice(64 * (hl % 2), 64 * (hl % 2) + 64)
                nc.sync.dma_start(gsb[0:64, hl, :], gtmp[sl, hl // 2, :])
                nc.sync.dma_start(qT[0:64, hl, :], qTp[sl, hl // 2, :])
                nc.sync.dma_start(kT[0:64, hl, :], kTp[sl, hl // 2, :])

        # ================= phase 2: attention =================
        if PHASES < 2:
            return
        with tc.tile_pool(name="sa", bufs=1, space="PSUM") as sap, \
             tc.tile_pool(name="sb", bufs=1, space="PSUM") as sbp, \
             tc.tile_pool(name="ov", bufs=2, space="PSUM") as ovp, \
             tc.tile_pool(name="pt", bufs=3) as ptp:

            for pr in range(2):
                for qb in range(NQB):
                    J = 2 * qb + 2
                    qs = slice(QBW * qb, QBW * qb + QBW)
                    od = {}
                    for pe in range(2):
                        od[pe] = ovp.tile([128, QBW], F32, tag="ov",
                                          name=f"ov{pe}")

                    ngroups = (J + SLOTS - 1) // SLOTS
                    for g in range(ngroups):
                        j0 = g * SLOTS
                        L = min(SLOTS, J - j0)
                        jd_hi = j0 + L - 1
                        sps = {0: sap.tile([128, SLOTS, QBW], F32, tag="sa", name="sA"),
                               1: sbp.tile([128, SLOTS, QBW], F32, tag="sb", name="sB")}
                        # scores: slot s holds jd = jd_hi - s (so the E-table
                        # group view has a positive slot stride)
                        for s in range(L):
                            jd = jd_hi - s
                            js = slice(128 * jd, 128 * jd + 128)
                            for pe in range(2):
                                hl = 2 * pr + pe
                                nc.tensor.matmul(
                                    sps[pe][:, s, :],
                                    kT[:, hl, js],
                                    qT[:, hl, qs],
                                    start=True, stop=True)
                        # exp + E multiply (grouped)
                        pts = {}
                        for pe in range(2):
                            hl = 2 * pr + pe
                            pt = ptp.tile([128, SLOTS, QBW], BF16, tag="pt", name=f"pt{pe}")
                            pts[pe] = pt
                            nc.scalar.activation(pt[:, :L, :], sps[pe][:, :L, :],
                                                 AF.Exp)
                            u0 = ETAB_OFF + QBW * qb - 128 * jd_hi
                            eview = _eap(etab_sb, hl * ETAB_U + u0,
                                         [[HG * ETAB_U, 128], [128, L], [1, QBW]])
                            nc.vector.tensor_tensor(pt[:, :L, :], pt[:, :L, :],
                                                    eview, ALU.mult)
                        # AV: full K=128 contraction per j-tile
                        for s in range(L):
                            jd = jd_hi - s
                            for pe in range(2):
                                hl = 2 * pr + pe
                                nc.tensor.matmul(
                                    od[pe][0:68, :],
                                    vv[:, hl, jd, 0:68],
                                    pts[pe][:, s, :],
                                    start=(jd == jd_hi if g == 0 else False),
                                    stop=(jd == 0 and g == ngroups - 1))

                    # evict: stash the denominator rows; gate-multiply the
                    # (unnormalized) o into the z tile; division by l happens
                    # at the out-projection eviction
                    for pe in range(2):
                        hl = 2 * pr + pe
                        nc.vector.tensor_add(lpack[64:68, qs],
                                             lpack[64:68, qs],
                                             od[pe][64:68, :])
                        nc.vector.tensor_mul(zt[0:64, hl, qs], od[pe][0:64, :],
                                             gsb[0:64, hl, qs])

        # ================= phase 3: out projection =================
        if PHASES < 3:
            return
        # y[n] = sum_h (z~_h @ wo_h) / l_h[n]; the division applies at psum
        # eviction with per-partition reciprocal columns obtained by PE-
        # transposing the packed l rows.
        with tc.tile_pool(name="yps", bufs=4, space="PSUM") as yp, \
             tc.tile_pool(name="lps", bufs=2, space="PSUM") as lpp, \
             tc.tile_pool(name="ysb", bufs=4) as ysp:
            for nt in range(16):
                nts = slice(128 * nt, 128 * nt + 128)
                lps = lpp.tile([128, 64], F32, tag="lps")
                nc.tensor.transpose(lps[:, :], lpack[64:128, nts],
                                    id_sb[64:128, :])
                nc.vector.reciprocal(rcol_sb[:, nt, :], lps[:, 0:HG])
                ya = ysp.tile([128, D], F32, tag="ya")
                yb = ysp.tile([128, D], F32, tag="yb")
                for hl in range(HG):
                    psy = yp.tile([128, D], F32, tag="psy", name=f"psy{hl}")
                    nc.tensor.matmul(psy[:, :], zt[0:64, hl, nts],
                                     wo_sb[0:64, hl, :],
                                     start=True, stop=True)
                    r = rcol_sb[:, nt, hl:hl + 1]
                    if hl == 0:
                        nc.scalar.activation(ya[:, :], psy[:, :], AF.Copy,
                                             scale=r)
                    elif hl == 1:
                        nc.vector.scalar_tensor_tensor(
                            ya[:, :], psy[:, :], r, ya[:, :],
                            op0=ALU.mult, op1=ALU.add)
                    elif hl == 2:
                        nc.scalar.activation(yb[:, :], psy[:, :], AF.Copy,
                                             scale=r)
                    else:
                        nc.vector.scalar_tensor_tensor(
                            yb[:, :], psy[:, :], r, yb[:, :],
                            op0=ALU.mult, op1=ALU.add)
                nc.sync.dma_start(y_d[nts, :], ya[:, :])
                nc.gpsimd.dma_start(y_d[nts, :], yb[:, :], accum_op=ALU.add)


def kernel(**inputs):
    global LAST_RESULT, _CACHED_NC
    in_maps = _prep_cores(inputs)
    if _CACHED_NC is None:
        _CACHED_NC = _build_nc()
    nc = _CACHED_NC
    res = run_bass_kernel_spmd(nc, in_maps, core_ids=list(range(NCORES)),
                               trace=TRACE)
    LAST_RESULT = res
    out_b = np.asarray(inputs["out_b"], dtype=np.float32)
    full = np.empty((B, N, D), dtype=np.float32)
    for b in range(B):
        full[b] = res.results[2 * b]["y"] + res.results[2 * b + 1]["y"] + out_b
    return full
